# revision 22
# baseline (speedup 1.0000x reference)
"""Distributed attention kernel for 8 TRN2 NeuronCores.

Problem: L=2048, B=2, E=256, H=8 heads, D=32 head-dim, fp32.

Sharding: DP2 over batch x sequence-parallel-4 over query positions.
Core c handles batch c//4, query rows [512*(c%4), 512*(c%4+1)), ALL 8
heads. k/v projections are redundantly computed per batch group (cheap)
and NO collective is needed: each core owns a disjoint output block.

Per-core pipeline:
  phase 0: DMA in bf16 x.T shards (k/v full batch, q slice) + weights.
  phase 1: k.T (all heads) = Wk^T x.T; q.T slice; v (natural layout) via
           a bf16 staging tile + strided SBUF->SBUF DMA into per-head
           [v|1] slots (ones column gives the softmax row-sum for free).
  phase 2: two passes of 4 heads; per (head, tk-pair): S.T = k.T^T q.T
           (bf16, f32 psum), exp on ScalarE with fused 1/sqrt(D) scale
           -> P.T bf16; PV uses P.T chunks as the STATIONARY operand and
           [v|1] as moving, so O lands in natural [tq, d] orientation with
           the softmax denominator Z per-partition (cheap reciprocal +
           tensor_scalar normalize); xbar DMA transposes (4 heads x 32
           dims packed) produce O.T for the projection off-PE. Each
           pass's projection contribution is accumulated early so only
           the second half sits on the tail.
  phase 3: bias + DMA straight to the output. No collective.
"""

import os
import sys

import numpy as np

for _p in ("/opt/trn_rl_repo",):
    if _p not in sys.path and os.path.isdir(_p):
        sys.path.insert(0, _p)

import ml_dtypes

import concourse.bass as bass
import concourse.bacc as bacc
import concourse.mybir as mybir
import concourse.tile as tile
from concourse.bass_utils import run_bass_kernel_spmd

dt = mybir.dt
F32 = dt.float32
BF16 = dt.bfloat16
AF = mybir.ActivationFunctionType
ALU = mybir.AluOpType
BF = ml_dtypes.bfloat16

L, B, E, H, D = 2048, 2, 256, 8, 32
SCALE = float(D) ** -0.5
NCORES = 8
SP = 4            # sequence-parallel ways
TQ = L // SP      # 512 query rows per core
NTK = L // 128    # 16 tk chunks
VW = H * (D + 1)  # v_buf cols per tk chunk: 8x [v_h | 1] = 264
NPASS = 2         # head passes (4 heads each)

_GRAPH = None


def _build_graph():
    nc = bacc.Bacc(
        "TRN2",
        target_bir_lowering=False,
        debug=False,
        enable_asserts=False,
        num_devices=NCORES,
    )

    xqt = nc.declare_dram_parameter("xqt", [E, TQ], BF16, isOutput=False).ap()
    xkt = nc.declare_dram_parameter("xkt", [E, L], BF16, isOutput=False).ap()
    xvt = nc.declare_dram_parameter("xvt", [E, L], BF16, isOutput=False).ap()
    wq = nc.declare_dram_parameter("wq", [E, E], BF16, isOutput=False).ap()
    wk = nc.declare_dram_parameter("wk", [E, E], BF16, isOutput=False).ap()
    wv = nc.declare_dram_parameter("wv", [E, E], BF16, isOutput=False).ap()
    wp = nc.declare_dram_parameter("wp", [E, E], BF16, isOutput=False).ap()
    bq = nc.declare_dram_parameter("bq", [1, E], F32, isOutput=False).ap()
    bk = nc.declare_dram_parameter("bk", [1, E], F32, isOutput=False).ap()
    bv = nc.declare_dram_parameter("bv", [1, E], F32, isOutput=False).ap()
    bp = nc.declare_dram_parameter("bp", [1, E], F32, isOutput=False).ap()
    out = nc.declare_dram_parameter("out", [TQ, E], F32, isOutput=True).ap()

    with tile.TileContext(nc) as tc:
        with (
            tc.tile_pool(name="persist", bufs=1) as pp,
            tc.tile_pool(name="pt", bufs=6) as ptp,
            tc.tile_pool(name="osb", bufs=2) as osbp,
            tc.tile_pool(name="rz", bufs=8) as rzp,
            tc.tile_pool(name="vstage", bufs=4) as vsp,
            tc.tile_pool(name="outsb", bufs=4) as outp,
            tc.tile_pool(name="st", bufs=2, space="PSUM") as stp,
            tc.tile_pool(name="ot", bufs=2, space="PSUM") as otp,
            tc.tile_pool(name="pj", bufs=2, space="PSUM") as pjp,
        ):
            # ---------- phase 0: loads ----------
            warm = pp.tile([1, 16], F32)
            nc.vector.memset(warm[:], 0.0)
            nc.scalar.activation(warm[:], warm[:], AF.Exp)

            # weights: tile [128, 2E]; slice e covers W rows [128e, 128e+128)
            w_sb = {}
            for name, wsrc in (("k", wk), ("q", wq), ("v", wv), ("p", wp)):
                t = pp.tile([128, 2 * E], BF16, tag=f"w{name}")
                # one DMA per W on the ScalarE HWDGE queue (parallel with
                # the x.T stream on SyncE): out free = (e, n), in = (e p) n
                nc.scalar.dma_start(
                    out=t[:].rearrange("p (e n) -> p e n", e=2),
                    in_=wsrc.rearrange("(e p) n -> p e n", p=128),
                )
                w_sb[name] = t

            # biases: bq/bk as per-partition columns [128, 2] (hc chunks);
            # bv/bp replicated across partitions
            bq_sb = pp.tile([128, 2], F32)
            nc.gpsimd.dma_start(
                out=bq_sb[:], in_=bq.rearrange("a (c p) -> p (a c)", p=128)
            )
            bk_sb = pp.tile([128, 2], F32)
            nc.gpsimd.dma_start(
                out=bk_sb[:], in_=bk.rearrange("a (c p) -> p (a c)", p=128)
            )
            bv_sb = pp.tile([128, E], F32)
            nc.gpsimd.dma_start(out=bv_sb[:], in_=bv.to_broadcast((128, E)))
            bp_sb = pp.tile([128, E], F32)
            nc.gpsimd.dma_start(out=bp_sb[:], in_=bp.to_broadcast((128, E)))
            # x.T loads AFTER weights (same HWDGE queue: weights must
            # land first so the first projections are not starved).
            # k first, then q, then v - matching first-use order.
            xk_sb = []
            for e in range(2):
                t = pp.tile([128, L], BF16, name=f"xkt{e}", tag=f"xkt{e}")
                for n in range(L // 512):
                    nc.sync.dma_start(
                        out=t[:, n * 512:(n + 1) * 512],
                        in_=xkt[e * 128:(e + 1) * 128, n * 512:(n + 1) * 512],
                    )
                xk_sb.append(t)
            xq_sb = []
            for e in range(2):
                t = pp.tile([128, TQ], BF16, name=f"xqt{e}", tag=f"xqt{e}")
                nc.scalar.dma_start(out=t[:], in_=xqt[e * 128:(e + 1) * 128, :])
                xq_sb.append(t)
            xv_sb = []
            for e in range(2):
                t = pp.tile([128, L], BF16, name=f"xvt{e}", tag=f"xvt{e}")
                for n in range(L // 512):
                    nc.sync.dma_start(
                        out=t[:, n * 512:(n + 1) * 512],
                        in_=xvt[e * 128:(e + 1) * 128, n * 512:(n + 1) * 512],
                    )
                xv_sb.append(t)

            # ---------- phase 1: projections ----------
            # k.T: [256 head-dims, 2048] as four [64, 2048] tiles
            # (2 heads per tile at partition bases 0/32 - PE requires
            # lhsT/rhs base partitions in {0, 32, 64})
            kT = [pp.tile([64, L], BF16, name=f"kT{pc}", tag=f"kT{pc}")
                  for pc in range(4)]
            for hc in range(2):
                for n in range(L // 512):
                    ps = pjp.tile([128, 512], F32, tag="pj")
                    for e in range(2):
                        nc.tensor.matmul(
                            ps[:],
                            w_sb["k"][:, e * E + hc * 128: e * E + (hc + 1) * 128],
                            xk_sb[e][:, n * 512:(n + 1) * 512],
                            start=(e == 0),
                            stop=(e == 1),
                        )
                    for half in range(2):
                        nc.vector.tensor_scalar_add(
                            kT[2 * hc + half][:, n * 512:(n + 1) * 512],
                            ps[half * 64:(half + 1) * 64, :],
                            bk_sb[half * 64:(half + 1) * 64, hc:hc + 1],
                        )

            # q.T slice: four [64, 512] tiles
            qT = [pp.tile([64, TQ], BF16, name=f"qT{pc}", tag=f"qT{pc}")
                  for pc in range(4)]
            for hc in range(2):
                ps = pjp.tile([128, 512], F32, tag="pj")
                for e in range(2):
                    nc.tensor.matmul(
                        ps[:],
                        w_sb["q"][:, e * E + hc * 128: e * E + (hc + 1) * 128],
                        xq_sb[e][:, :],
                        start=(e == 0),
                        stop=(e == 1),
                    )
                for half in range(2):
                    nc.vector.tensor_scalar_add(
                        qT[2 * hc + half][:, :],
                        ps[half * 64:(half + 1) * 64, :],
                        bq_sb[half * 64:(half + 1) * 64, hc:hc + 1],
                    )

            # v_buf: per tk chunk, 8x [v_h (32) | 1] slots
            v_buf = pp.tile([128, NTK * VW], BF16)
            nc.gpsimd.memset(v_buf[:], 1.0)
            for t in range(NTK):
                ps = pjp.tile([128, E], F32, tag="pj")
                for e in range(2):
                    nc.tensor.matmul(
                        ps[:],
                        xv_sb[e][:, t * 128:(t + 1) * 128],
                        w_sb["v"][:, e * E:(e + 1) * E],
                        start=(e == 0),
                        stop=(e == 1),
                    )
                vs = vsp.tile([128, E], BF16, tag="vstage")
                nc.vector.tensor_tensor(vs[:], ps[:], bv_sb[:], ALU.add)
                # scatter the 8 heads' 32-col blocks into the [v|1] slots
                nc.sync.dma_start(
                    out=v_buf[:, t * VW:(t + 1) * VW].rearrange(
                        "p (h w) -> p h w", h=H
                    )[:, :, 0:D],
                    in_=vs[:].rearrange("p (h d) -> p h d", h=H),
                )

            # proj psum: two [128, 512] tiles hold the four [128, 256]
            # tq-chunk partials across both passes
            pjt = [pjp.tile([128, 2 * E], F32, name=f"pjt{i}", tag="pj")
                   for i in range(2)]

            # ---------- phase 2: attention (2 passes of 4 heads) ----------
            # PV uses P.T chunks as the STATIONARY operand and [v|1] as the
            # moving operand, so O lands in natural [tq, d] orientation with
            # the softmax denominator Z as a per-partition column. The O.T
            # needed by the projection comes from xbar DMA transposes.
            o_sb = []
            for p in range(NPASS):
                osb = osbp.tile([128, TQ], BF16, tag="osb")
                o_sb.append(osb)
                # o_nat: [tq(4x128), 4 heads x 32] natural-orientation output
                onat = osbp.tile([128, TQ], BF16, tag="onat")
                for u in range(4):  # heads within pass
                    h = p * 4 + u
                    hc, hr = h // 2, (h % 2) * D
                    # po: per-tqc [O_h | Z] blocks: [128, 4*33]
                    po = otp.tile([128, 4 * (D + 1)], F32, tag="po")
                    for g in range(NTK // 2):
                        st = stp.tile([128, 1024], F32, tag="st")
                        for i in range(2):
                            tk = 2 * g + i
                            nc.tensor.matmul(
                                st[:, i * 512:(i + 1) * 512],
                                kT[hc][hr:hr + D, tk * 128:(tk + 1) * 128],
                                qT[hc][hr:hr + D, :],
                                start=True,
                                stop=True,
                            )
                        pt = ptp.tile([128, 1024], BF16, tag="pt")
                        nc.scalar.activation(pt[:], st[:], AF.Exp, scale=SCALE)
                        for i in range(2):
                            tk = 2 * g + i
                            for m in range(4):  # tq 128-chunks
                                # start=True zeroes the whole 2KB psum bank
                                # row, so only the first matmul starts the
                                # group and only the last one stops it.
                                nc.tensor.matmul(
                                    po[:, m * (D + 1):(m + 1) * (D + 1)],
                                    pt[:, i * 512 + m * 128: i * 512 + (m + 1) * 128],
                                    v_buf[:, tk * VW + h * (D + 1): tk * VW + (h + 1) * (D + 1)],
                                    start=(g == 0 and i == 0 and m == 0),
                                    stop=(g == NTK // 2 - 1 and i == 1 and m == 3),
                                    skip_group_check=True,
                                )

                    # normalize head h: per-partition 1/Z then scalar mul
                    rz = rzp.tile([128, 4], F32, tag="rz")
                    nc.vector.reciprocal(
                        rz[:], po[:].rearrange("p (m w) -> p m w", w=D + 1)[:, :, D]
                    )
                    for m in range(4):
                        nc.vector.tensor_scalar_mul(
                            onat[:, m * 128 + u * D: m * 128 + (u + 1) * D],
                            po[:, m * (D + 1): m * (D + 1) + D],
                            rz[:, m:m + 1],
                        )

                # o_sb[:, m*128:+128] = transpose(o_nat[:, m*128:+128]):
                # [tq 128, 4 heads x 32] -> [4 heads x 32, tq 128]
                for m in range(4):
                        eng = nc.sync if m % 2 == 0 else nc.scalar
                        eng.dma_start_transpose(
                            osb[:, m * 128:(m + 1) * 128],
                            onat[:, m * 128:(m + 1) * 128],
                        )

                # accumulate this pass's projection contribution
                # (pass 0's matmuls overlap pass 1's attention compute)
                for m in range(4):
                    nc.tensor.matmul(
                        pjt[m // 2][:, (m % 2) * E:(m % 2 + 1) * E],
                        osb[:, m * 128:(m + 1) * 128],
                        w_sb["p"][:, p * E:(p + 1) * E],
                        start=(p == 0 and m % 2 == 0),
                        stop=(p == NPASS - 1 and m % 2 == 1),
                        skip_group_check=True,
                    )

            # ---------- phase 3: bias + DMA out (proj accumulated
            # inside the pass loop above) ----------
            for m in range(TQ // 128):
                ob = outp.tile([128, E], F32, tag="outsb")
                nc.vector.tensor_tensor(
                    ob[:], pjt[m // 2][:, (m % 2) * E:(m % 2 + 1) * E],
                    bp_sb[:], ALU.add,
                )
                eng = nc.sync if m % 2 == 0 else nc.scalar
                eng.dma_start(
                    out=out[m * 128:(m + 1) * 128, :], in_=ob[:]
                )

    return nc


def get_graph():
    global _GRAPH
    if _GRAPH is None:
        nc = _build_graph()
        nc.compile()
        _GRAPH = nc
    return _GRAPH


def make_in_maps(query, key_, value, Wq, bq, Wk, bk, Wv, bv, Wp, bp):
    query = np.asarray(query, np.float32)
    key_ = np.asarray(key_, np.float32)
    value = np.asarray(value, np.float32)
    Wq, Wk, Wv, Wp = (np.asarray(w, np.float32) for w in (Wq, Wk, Wv, Wp))
    bq, bk, bv, bp = (np.asarray(b_, np.float32) for b_ in (bq, bk, bv, bp))

    wq_b = np.ascontiguousarray(Wq).astype(BF)
    wk_b = np.ascontiguousarray(Wk).astype(BF)
    wv_b = np.ascontiguousarray(Wv).astype(BF)
    wp_b = np.ascontiguousarray(Wp).astype(BF)
    xt = {}
    for b in range(B):
        xt[("q", b)] = np.ascontiguousarray(query[:, b, :].T).astype(BF)
        xt[("k", b)] = np.ascontiguousarray(key_[:, b, :].T).astype(BF)
        xt[("v", b)] = np.ascontiguousarray(value[:, b, :].T).astype(BF)

    in_maps = []
    for c in range(NCORES):
        b = c // SP
        p = c % SP
        m = {
            "xqt": np.ascontiguousarray(xt[("q", b)][:, p * TQ:(p + 1) * TQ]),
            "xkt": xt[("k", b)],
            "xvt": xt[("v", b)],
            "wq": wq_b,
            "wk": wk_b,
            "wv": wv_b,
            "wp": wp_b,
            "bq": bq.reshape(1, E).copy(),
            "bk": bk.reshape(1, E).copy(),
            "bv": bv.reshape(1, E).copy(),
            "bp": bp.reshape(1, E).copy(),
        }
        in_maps.append(m)
    return in_maps


def assemble(results):
    out_full = np.empty((L, B, E), np.float32)
    for c in range(NCORES):
        b = c // SP
        p = c % SP
        out_full[p * TQ:(p + 1) * TQ, b, :] = results[c]["out"]
    return out_full


def run(inputs, trace=False, **kw):
    nc = get_graph()
    in_maps = make_in_maps(**inputs)
    res = run_bass_kernel_spmd(
        nc, in_maps, core_ids=list(range(NCORES)), trace=trace, **kw
    )
    return res


def kernel(**inputs):
    res = run(inputs, trace=False)
    return assemble(res.results)


# revision 23
# speedup vs baseline: 1.0045x; 1.0045x over previous
"""Distributed attention kernel for 8 TRN2 NeuronCores.

Problem: L=2048, B=2, E=256, H=8 heads, D=32 head-dim, fp32.

Sharding: DP2 over batch x sequence-parallel-4 over query positions.
Core c handles batch c//4, query rows [512*(c%4), 512*(c%4+1)), ALL 8
heads. k/v projections are redundantly computed per batch group (cheap)
and NO collective is needed: each core owns a disjoint output block.

Per-core pipeline:
  phase 0: DMA in bf16 x.T shards (k/v full batch, q slice) + weights.
  phase 1: k.T (all heads) = Wk^T x.T; q.T slice; v (natural layout) via
           a bf16 staging tile + strided SBUF->SBUF DMA into per-head
           [v|1] slots (ones column gives the softmax row-sum for free).
  phase 2: two passes of 4 heads; per (head, tk-pair): S.T = k.T^T q.T
           (bf16, f32 psum), exp on ScalarE with fused 1/sqrt(D) scale
           -> P.T bf16; PV uses P.T chunks as the STATIONARY operand and
           [v|1] as moving, so O lands in natural [tq, d] orientation with
           the softmax denominator Z per-partition (cheap reciprocal +
           tensor_scalar normalize); xbar DMA transposes (4 heads x 32
           dims packed) produce O.T for the projection off-PE. Each
           pass's projection contribution is accumulated early so only
           the second half sits on the tail.
  phase 3: bias + DMA straight to the output. No collective.
"""

import os
import sys

import numpy as np

for _p in ("/opt/trn_rl_repo",):
    if _p not in sys.path and os.path.isdir(_p):
        sys.path.insert(0, _p)

import ml_dtypes

import concourse.bass as bass
import concourse.bacc as bacc
import concourse.mybir as mybir
import concourse.tile as tile
from concourse.bass_utils import run_bass_kernel_spmd

dt = mybir.dt
F32 = dt.float32
BF16 = dt.bfloat16
AF = mybir.ActivationFunctionType
ALU = mybir.AluOpType
BF = ml_dtypes.bfloat16

L, B, E, H, D = 2048, 2, 256, 8, 32
SCALE = float(D) ** -0.5
NCORES = 8
SP = 4            # sequence-parallel ways
TQ = L // SP      # 512 query rows per core
NTK = L // 128    # 16 tk chunks
VW = H * (D + 1)  # v_buf cols per tk chunk: 8x [v_h | 1] = 264
NPASS = 2         # head passes (4 heads each)

_GRAPH = None


def _build_graph():
    nc = bacc.Bacc(
        "TRN2",
        target_bir_lowering=False,
        debug=False,
        enable_asserts=False,
        num_devices=NCORES,
    )

    xqt = nc.declare_dram_parameter("xqt", [E, TQ], BF16, isOutput=False).ap()
    xkt = nc.declare_dram_parameter("xkt", [E, L], BF16, isOutput=False).ap()
    xvt = nc.declare_dram_parameter("xvt", [E, L], BF16, isOutput=False).ap()
    wq = nc.declare_dram_parameter("wq", [E, E], BF16, isOutput=False).ap()
    wk = nc.declare_dram_parameter("wk", [E, E], BF16, isOutput=False).ap()
    wv = nc.declare_dram_parameter("wv", [E, E], BF16, isOutput=False).ap()
    wp = nc.declare_dram_parameter("wp", [E, E], BF16, isOutput=False).ap()
    bq = nc.declare_dram_parameter("bq", [1, E], F32, isOutput=False).ap()
    bk = nc.declare_dram_parameter("bk", [1, E], F32, isOutput=False).ap()
    bv = nc.declare_dram_parameter("bv", [1, E], F32, isOutput=False).ap()
    bp = nc.declare_dram_parameter("bp", [1, E], F32, isOutput=False).ap()
    out = nc.declare_dram_parameter("out", [TQ, E], F32, isOutput=True).ap()

    with tile.TileContext(nc) as tc:
        with (
            tc.tile_pool(name="persist", bufs=1) as pp,
            tc.tile_pool(name="pt", bufs=10) as ptp,
            tc.tile_pool(name="osb", bufs=3) as osbp,
            tc.tile_pool(name="rz", bufs=8) as rzp,
            tc.tile_pool(name="vstage", bufs=6) as vsp,
            tc.tile_pool(name="outsb", bufs=4) as outp,
            tc.tile_pool(name="st", bufs=2, space="PSUM") as stp,
            tc.tile_pool(name="ot", bufs=2, space="PSUM") as otp,
            tc.tile_pool(name="pj", bufs=2, space="PSUM") as pjp,
        ):
            # ---------- phase 0: loads ----------
            warm = pp.tile([1, 16], F32)
            nc.vector.memset(warm[:], 0.0)
            nc.scalar.activation(warm[:], warm[:], AF.Exp)

            # weights: tile [128, 2E]; slice e covers W rows [128e, 128e+128)
            w_sb = {}
            for name, wsrc in (("k", wk), ("q", wq), ("v", wv), ("p", wp)):
                t = pp.tile([128, 2 * E], BF16, tag=f"w{name}")
                # one DMA per W on the ScalarE HWDGE queue (parallel with
                # the x.T stream on SyncE): out free = (e, n), in = (e p) n
                nc.scalar.dma_start(
                    out=t[:].rearrange("p (e n) -> p e n", e=2),
                    in_=wsrc.rearrange("(e p) n -> p e n", p=128),
                )
                w_sb[name] = t

            # biases: bq/bk as per-partition columns [128, 2] (hc chunks);
            # bv/bp replicated across partitions
            bq_sb = pp.tile([128, 2], F32)
            nc.gpsimd.dma_start(
                out=bq_sb[:], in_=bq.rearrange("a (c p) -> p (a c)", p=128)
            )
            bk_sb = pp.tile([128, 2], F32)
            nc.gpsimd.dma_start(
                out=bk_sb[:], in_=bk.rearrange("a (c p) -> p (a c)", p=128)
            )
            bv_sb = pp.tile([128, E], F32)
            nc.gpsimd.dma_start(out=bv_sb[:], in_=bv.to_broadcast((128, E)))
            bp_sb = pp.tile([128, E], F32)
            nc.gpsimd.dma_start(out=bp_sb[:], in_=bp.to_broadcast((128, E)))
            # x.T loads AFTER weights (same HWDGE queue: weights must
            # land first so the first projections are not starved).
            # k first, then q, then v - matching first-use order.
            xk_sb = []
            for e in range(2):
                t = pp.tile([128, L], BF16, name=f"xkt{e}", tag=f"xkt{e}")
                for n in range(L // 512):
                    nc.sync.dma_start(
                        out=t[:, n * 512:(n + 1) * 512],
                        in_=xkt[e * 128:(e + 1) * 128, n * 512:(n + 1) * 512],
                    )
                xk_sb.append(t)
            xq_sb = []
            for e in range(2):
                t = pp.tile([128, TQ], BF16, name=f"xqt{e}", tag=f"xqt{e}")
                nc.scalar.dma_start(out=t[:], in_=xqt[e * 128:(e + 1) * 128, :])
                xq_sb.append(t)
            xv_sb = []
            for e in range(2):
                t = pp.tile([128, L], BF16, name=f"xvt{e}", tag=f"xvt{e}")
                for n in range(L // 512):
                    nc.sync.dma_start(
                        out=t[:, n * 512:(n + 1) * 512],
                        in_=xvt[e * 128:(e + 1) * 128, n * 512:(n + 1) * 512],
                    )
                xv_sb.append(t)

            # ---------- phase 1: projections ----------
            # k.T: [256 head-dims, 2048] as four [64, 2048] tiles
            # (2 heads per tile at partition bases 0/32 - PE requires
            # lhsT/rhs base partitions in {0, 32, 64})
            kT = [pp.tile([64, L], BF16, name=f"kT{pc}", tag=f"kT{pc}")
                  for pc in range(4)]
            for hc in range(2):
                for n in range(L // 512):
                    ps = pjp.tile([128, 512], F32, tag="pj")
                    for e in range(2):
                        nc.tensor.matmul(
                            ps[:],
                            w_sb["k"][:, e * E + hc * 128: e * E + (hc + 1) * 128],
                            xk_sb[e][:, n * 512:(n + 1) * 512],
                            start=(e == 0),
                            stop=(e == 1),
                        )
                    for half in range(2):
                        nc.vector.tensor_scalar_add(
                            kT[2 * hc + half][:, n * 512:(n + 1) * 512],
                            ps[half * 64:(half + 1) * 64, :],
                            bk_sb[half * 64:(half + 1) * 64, hc:hc + 1],
                        )

            # q.T slice: four [64, 512] tiles
            qT = [pp.tile([64, TQ], BF16, name=f"qT{pc}", tag=f"qT{pc}")
                  for pc in range(4)]
            for hc in range(2):
                ps = pjp.tile([128, 512], F32, tag="pj")
                for e in range(2):
                    nc.tensor.matmul(
                        ps[:],
                        w_sb["q"][:, e * E + hc * 128: e * E + (hc + 1) * 128],
                        xq_sb[e][:, :],
                        start=(e == 0),
                        stop=(e == 1),
                    )
                for half in range(2):
                    nc.vector.tensor_scalar_add(
                        qT[2 * hc + half][:, :],
                        ps[half * 64:(half + 1) * 64, :],
                        bq_sb[half * 64:(half + 1) * 64, hc:hc + 1],
                    )

            # v_buf: per tk chunk, 8x [v_h (32) | 1] slots
            v_buf = pp.tile([128, NTK * VW], BF16)
            nc.gpsimd.memset(v_buf[:], 1.0)
            for t in range(NTK):
                ps = pjp.tile([128, E], F32, tag="pj")
                for e in range(2):
                    nc.tensor.matmul(
                        ps[:],
                        xv_sb[e][:, t * 128:(t + 1) * 128],
                        w_sb["v"][:, e * E:(e + 1) * E],
                        start=(e == 0),
                        stop=(e == 1),
                    )
                vs = vsp.tile([128, E], BF16, tag="vstage")
                nc.vector.tensor_tensor(vs[:], ps[:], bv_sb[:], ALU.add)
                # scatter the 8 heads' 32-col blocks into the [v|1] slots
                nc.sync.dma_start(
                    out=v_buf[:, t * VW:(t + 1) * VW].rearrange(
                        "p (h w) -> p h w", h=H
                    )[:, :, 0:D],
                    in_=vs[:].rearrange("p (h d) -> p h d", h=H),
                )

            # proj psum: two [128, 512] tiles hold the four [128, 256]
            # tq-chunk partials across both passes
            pjt = [pjp.tile([128, 2 * E], F32, name=f"pjt{i}", tag="pj")
                   for i in range(2)]

            # ---------- phase 2: attention (2 passes of 4 heads) ----------
            # PV uses P.T chunks as the STATIONARY operand and [v|1] as the
            # moving operand, so O lands in natural [tq, d] orientation with
            # the softmax denominator Z as a per-partition column. The O.T
            # needed by the projection comes from xbar DMA transposes.
            o_sb = []
            for p in range(NPASS):
                osb = osbp.tile([128, TQ], BF16, tag="osb")
                o_sb.append(osb)
                # o_nat: [tq(4x128), 4 heads x 32] natural-orientation output
                onat = osbp.tile([128, TQ], BF16, tag="onat")
                for u in range(4):  # heads within pass
                    h = p * 4 + u
                    hc, hr = h // 2, (h % 2) * D
                    # po: per-tqc [O_h | Z] blocks: [128, 4*33]
                    po = otp.tile([128, 4 * (D + 1)], F32, tag="po")
                    for g in range(NTK // 2):
                        st = stp.tile([128, 1024], F32, tag="st")
                        for i in range(2):
                            tk = 2 * g + i
                            nc.tensor.matmul(
                                st[:, i * 512:(i + 1) * 512],
                                kT[hc][hr:hr + D, tk * 128:(tk + 1) * 128],
                                qT[hc][hr:hr + D, :],
                                start=True,
                                stop=True,
                            )
                        pt = ptp.tile([128, 1024], BF16, tag="pt")
                        nc.scalar.activation(pt[:], st[:], AF.Exp, scale=SCALE)
                        for i in range(2):
                            tk = 2 * g + i
                            for m in range(4):  # tq 128-chunks
                                # start=True zeroes the whole 2KB psum bank
                                # row, so only the first matmul starts the
                                # group and only the last one stops it.
                                nc.tensor.matmul(
                                    po[:, m * (D + 1):(m + 1) * (D + 1)],
                                    pt[:, i * 512 + m * 128: i * 512 + (m + 1) * 128],
                                    v_buf[:, tk * VW + h * (D + 1): tk * VW + (h + 1) * (D + 1)],
                                    start=(g == 0 and i == 0 and m == 0),
                                    stop=(g == NTK // 2 - 1 and i == 1 and m == 3),
                                    skip_group_check=True,
                                )

                    # normalize head h: per-partition 1/Z then scalar mul
                    rz = rzp.tile([128, 4], F32, tag="rz")
                    nc.vector.reciprocal(
                        rz[:], po[:].rearrange("p (m w) -> p m w", w=D + 1)[:, :, D]
                    )
                    for m in range(4):
                        nc.vector.tensor_scalar_mul(
                            onat[:, m * 128 + u * D: m * 128 + (u + 1) * D],
                            po[:, m * (D + 1): m * (D + 1) + D],
                            rz[:, m:m + 1],
                        )

                # o_sb[:, m*128:+128] = transpose(o_nat[:, m*128:+128]):
                # [tq 128, 4 heads x 32] -> [4 heads x 32, tq 128]
                for m in range(4):
                        eng = nc.sync if m % 2 == 0 else nc.scalar
                        eng.dma_start_transpose(
                            osb[:, m * 128:(m + 1) * 128],
                            onat[:, m * 128:(m + 1) * 128],
                        )

                # accumulate this pass's projection contribution
                # (pass 0's matmuls overlap pass 1's attention compute)
                for m in range(4):
                    nc.tensor.matmul(
                        pjt[m // 2][:, (m % 2) * E:(m % 2 + 1) * E],
                        osb[:, m * 128:(m + 1) * 128],
                        w_sb["p"][:, p * E:(p + 1) * E],
                        start=(p == 0 and m % 2 == 0),
                        stop=(p == NPASS - 1 and m % 2 == 1),
                        skip_group_check=True,
                    )

            # ---------- phase 3: bias + DMA out (proj accumulated
            # inside the pass loop above) ----------
            for m in range(TQ // 128):
                ob = outp.tile([128, E], F32, tag="outsb")
                nc.vector.tensor_tensor(
                    ob[:], pjt[m // 2][:, (m % 2) * E:(m % 2 + 1) * E],
                    bp_sb[:], ALU.add,
                )
                eng = nc.sync if m % 2 == 0 else nc.scalar
                eng.dma_start(
                    out=out[m * 128:(m + 1) * 128, :], in_=ob[:]
                )

    return nc


def get_graph():
    global _GRAPH
    if _GRAPH is None:
        nc = _build_graph()
        nc.compile()
        _GRAPH = nc
    return _GRAPH


def make_in_maps(query, key_, value, Wq, bq, Wk, bk, Wv, bv, Wp, bp):
    query = np.asarray(query, np.float32)
    key_ = np.asarray(key_, np.float32)
    value = np.asarray(value, np.float32)
    Wq, Wk, Wv, Wp = (np.asarray(w, np.float32) for w in (Wq, Wk, Wv, Wp))
    bq, bk, bv, bp = (np.asarray(b_, np.float32) for b_ in (bq, bk, bv, bp))

    wq_b = np.ascontiguousarray(Wq).astype(BF)
    wk_b = np.ascontiguousarray(Wk).astype(BF)
    wv_b = np.ascontiguousarray(Wv).astype(BF)
    wp_b = np.ascontiguousarray(Wp).astype(BF)
    xt = {}
    for b in range(B):
        xt[("q", b)] = np.ascontiguousarray(query[:, b, :].T).astype(BF)
        xt[("k", b)] = np.ascontiguousarray(key_[:, b, :].T).astype(BF)
        xt[("v", b)] = np.ascontiguousarray(value[:, b, :].T).astype(BF)

    in_maps = []
    for c in range(NCORES):
        b = c // SP
        p = c % SP
        m = {
            "xqt": np.ascontiguousarray(xt[("q", b)][:, p * TQ:(p + 1) * TQ]),
            "xkt": xt[("k", b)],
            "xvt": xt[("v", b)],
            "wq": wq_b,
            "wk": wk_b,
            "wv": wv_b,
            "wp": wp_b,
            "bq": bq.reshape(1, E).copy(),
            "bk": bk.reshape(1, E).copy(),
            "bv": bv.reshape(1, E).copy(),
            "bp": bp.reshape(1, E).copy(),
        }
        in_maps.append(m)
    return in_maps


def assemble(results):
    out_full = np.empty((L, B, E), np.float32)
    for c in range(NCORES):
        b = c // SP
        p = c % SP
        out_full[p * TQ:(p + 1) * TQ, b, :] = results[c]["out"]
    return out_full


def run(inputs, trace=False, **kw):
    nc = get_graph()
    in_maps = make_in_maps(**inputs)
    res = run_bass_kernel_spmd(
        nc, in_maps, core_ids=list(range(NCORES)), trace=trace, **kw
    )
    return res


def kernel(**inputs):
    res = run(inputs, trace=False)
    return assemble(res.results)


# revision 26
# speedup vs baseline: 1.0158x; 1.0112x over previous
"""Distributed attention kernel for 8 TRN2 NeuronCores.

Problem: L=2048, B=2, E=256, H=8 heads, D=32 head-dim, fp32.

Sharding: DP2 over batch x sequence-parallel-4 over query positions.
Core c handles batch c//4, query rows [512*(c%4), 512*(c%4+1)), ALL 8
heads. k/v projections are redundantly computed per batch group (cheap)
and NO collective is needed: each core owns a disjoint output block.

Per-core pipeline:
  phase 0: DMA in bf16 x.T shards (k/v full batch, q slice) + weights.
  phase 1: k.T (all heads) = Wk^T x.T; q.T slice; v (natural layout) via
           a bf16 staging tile + strided SBUF->SBUF DMA into per-head
           [v|1] slots (ones column gives the softmax row-sum for free).
  phase 2: two passes of 4 heads; per (head, tk-pair): S.T = k.T^T q.T
           (bf16, f32 psum), exp on ScalarE with fused 1/sqrt(D) scale
           -> P.T bf16; PV uses P.T chunks as the STATIONARY operand and
           [v|1] as moving, so O lands in natural [tq, d] orientation with
           the softmax denominator Z per-partition (cheap reciprocal +
           tensor_scalar normalize); xbar DMA transposes (4 heads x 32
           dims packed) produce O.T for the projection off-PE. Each
           pass's projection contribution is accumulated early so only
           the second half sits on the tail.
  phase 3: bias + DMA straight to the output. No collective.
"""

import os
import sys

import numpy as np

for _p in ("/opt/trn_rl_repo",):
    if _p not in sys.path and os.path.isdir(_p):
        sys.path.insert(0, _p)

import ml_dtypes

import concourse.bass as bass
import concourse.bacc as bacc
import concourse.mybir as mybir
import concourse.tile as tile
from concourse.bass_utils import run_bass_kernel_spmd

dt = mybir.dt
F32 = dt.float32
BF16 = dt.bfloat16
AF = mybir.ActivationFunctionType
ALU = mybir.AluOpType
BF = ml_dtypes.bfloat16

L, B, E, H, D = 2048, 2, 256, 8, 32
SCALE = float(D) ** -0.5
NCORES = 8
SP = 4            # sequence-parallel ways
TQ = L // SP      # 512 query rows per core
NTK = L // 128    # 16 tk chunks
VW = H * (D + 1)  # v_buf cols per tk chunk: 8x [v_h | 1] = 264
NPASS = 2         # head passes (4 heads each)

_GRAPH = None


def _build_graph():
    nc = bacc.Bacc(
        "TRN2",
        target_bir_lowering=False,
        debug=False,
        enable_asserts=False,
        num_devices=NCORES,
    )

    xqt = nc.declare_dram_parameter("xqt", [E, TQ], BF16, isOutput=False).ap()
    xkt = nc.declare_dram_parameter("xkt", [E, L], BF16, isOutput=False).ap()
    xvt = nc.declare_dram_parameter("xvt", [E, L], BF16, isOutput=False).ap()
    wq = nc.declare_dram_parameter("wq", [E, E], BF16, isOutput=False).ap()
    wk = nc.declare_dram_parameter("wk", [E, E], BF16, isOutput=False).ap()
    wv = nc.declare_dram_parameter("wv", [E, E], BF16, isOutput=False).ap()
    wp = nc.declare_dram_parameter("wp", [E, E], BF16, isOutput=False).ap()
    bq = nc.declare_dram_parameter("bq", [1, E], F32, isOutput=False).ap()
    bk = nc.declare_dram_parameter("bk", [1, E], F32, isOutput=False).ap()
    bv = nc.declare_dram_parameter("bv", [1, E], F32, isOutput=False).ap()
    bp = nc.declare_dram_parameter("bp", [1, E], F32, isOutput=False).ap()
    out = nc.declare_dram_parameter("out", [TQ, E], F32, isOutput=True).ap()

    with tile.TileContext(nc) as tc:
        with (
            tc.tile_pool(name="persist", bufs=1) as pp,
            tc.tile_pool(name="pt", bufs=10) as ptp,
            tc.tile_pool(name="osb", bufs=3) as osbp,
            tc.tile_pool(name="rz", bufs=8) as rzp,
            tc.tile_pool(name="vstage", bufs=6) as vsp,
            tc.tile_pool(name="outsb", bufs=4) as outp,
            tc.tile_pool(name="st", bufs=2, space="PSUM") as stp,
            tc.tile_pool(name="ot", bufs=2, space="PSUM") as otp,
            tc.tile_pool(name="pj", bufs=2, space="PSUM") as pjp,
        ):
            # ---------- phase 0: loads ----------
            warm = pp.tile([1, 16], F32)
            nc.vector.memset(warm[:], 0.0)
            nc.scalar.activation(warm[:], warm[:], AF.Exp)

            # weights: tile [128, 2E]; slice e covers W rows [128e, 128e+128)
            w_sb = {}

            def load_w(name, wsrc):
                t = pp.tile([128, 2 * E], BF16, name=f"w{name}", tag=f"w{name}")
                # one DMA per W on the ScalarE HWDGE queue (parallel with
                # the x.T stream on SyncE): out free = (e, n), in = (e p) n
                nc.scalar.dma_start(
                    out=t[:].rearrange("p (e n) -> p e n", e=2),
                    in_=wsrc.rearrange("(e p) n -> p e n", p=128),
                )
                w_sb[name] = t

            load_w("k", wk)
            load_w("q", wq)

            # biases: bq/bk as per-partition columns [128, 2] (hc chunks);
            # bv/bp replicated across partitions
            bq_sb = pp.tile([128, 2], F32)
            nc.gpsimd.dma_start(
                out=bq_sb[:], in_=bq.rearrange("a (c p) -> p (a c)", p=128)
            )
            bk_sb = pp.tile([128, 2], F32)
            nc.gpsimd.dma_start(
                out=bk_sb[:], in_=bk.rearrange("a (c p) -> p (a c)", p=128)
            )
            bv_sb = pp.tile([128, E], F32)
            nc.gpsimd.dma_start(out=bv_sb[:], in_=bv.to_broadcast((128, E)))
            bp_sb = pp.tile([128, E], F32)
            nc.gpsimd.dma_start(out=bp_sb[:], in_=bp.to_broadcast((128, E)))
            # x.T loads AFTER weights (same HWDGE queue: weights must
            # land first so the first projections are not starved).
            # k first, then q, then v - matching first-use order.
            xk_sb = [
                pp.tile([128, L], BF16, name=f"xkt{e}", tag=f"xkt{e}")
                for e in range(2)
            ]
            for n in range(L // 512):
                for e in range(2):
                    nc.sync.dma_start(
                        out=xk_sb[e][:, n * 512:(n + 1) * 512],
                        in_=xkt[e * 128:(e + 1) * 128, n * 512:(n + 1) * 512],
                    )
            xq_sb = []
            for e in range(2):
                t = pp.tile([128, TQ], BF16, name=f"xqt{e}", tag=f"xqt{e}")
                nc.scalar.dma_start(out=t[:], in_=xqt[e * 128:(e + 1) * 128, :])
                xq_sb.append(t)
            load_w("v", wv)
            load_w("p", wp)
            xv_sb = [
                pp.tile([128, L], BF16, name=f"xvt{e}", tag=f"xvt{e}")
                for e in range(2)
            ]
            for n in range(L // 512):
                for e in range(2):
                    nc.sync.dma_start(
                        out=xv_sb[e][:, n * 512:(n + 1) * 512],
                        in_=xvt[e * 128:(e + 1) * 128, n * 512:(n + 1) * 512],
                    )

            # ---------- phase 1: projections ----------
            # k.T: [256 head-dims, 2048] as four [64, 2048] tiles
            # (2 heads per tile at partition bases 0/32 - PE requires
            # lhsT/rhs base partitions in {0, 32, 64})
            kT = [pp.tile([64, L], BF16, name=f"kT{pc}", tag=f"kT{pc}")
                  for pc in range(4)]
            for hc in range(2):
                for n in range(L // 512):
                    ps = pjp.tile([128, 512], F32, tag="pj")
                    for e in range(2):
                        nc.tensor.matmul(
                            ps[:],
                            w_sb["k"][:, e * E + hc * 128: e * E + (hc + 1) * 128],
                            xk_sb[e][:, n * 512:(n + 1) * 512],
                            start=(e == 0),
                            stop=(e == 1),
                        )
                    for half in range(2):
                        nc.vector.tensor_scalar_add(
                            kT[2 * hc + half][:, n * 512:(n + 1) * 512],
                            ps[half * 64:(half + 1) * 64, :],
                            bk_sb[half * 64:(half + 1) * 64, hc:hc + 1],
                        )

            # q.T slice: four [64, 512] tiles
            qT = [pp.tile([64, TQ], BF16, name=f"qT{pc}", tag=f"qT{pc}")
                  for pc in range(4)]
            for hc in range(2):
                ps = pjp.tile([128, 512], F32, tag="pj")
                for e in range(2):
                    nc.tensor.matmul(
                        ps[:],
                        w_sb["q"][:, e * E + hc * 128: e * E + (hc + 1) * 128],
                        xq_sb[e][:, :],
                        start=(e == 0),
                        stop=(e == 1),
                    )
                for half in range(2):
                    nc.vector.tensor_scalar_add(
                        qT[2 * hc + half][:, :],
                        ps[half * 64:(half + 1) * 64, :],
                        bq_sb[half * 64:(half + 1) * 64, hc:hc + 1],
                    )

            # v_buf: per tk chunk, 8x [v_h (32) | 1] slots
            v_buf = pp.tile([128, NTK * VW], BF16)
            nc.gpsimd.memset(v_buf[:], 1.0)
            for t in range(NTK):
                ps = pjp.tile([128, E], F32, tag="pj")
                for e in range(2):
                    nc.tensor.matmul(
                        ps[:],
                        xv_sb[e][:, t * 128:(t + 1) * 128],
                        w_sb["v"][:, e * E:(e + 1) * E],
                        start=(e == 0),
                        stop=(e == 1),
                    )
                vs = vsp.tile([128, E], BF16, tag="vstage")
                nc.vector.tensor_tensor(vs[:], ps[:], bv_sb[:], ALU.add)
                # scatter the 8 heads' 32-col blocks into the [v|1] slots
                nc.sync.dma_start(
                    out=v_buf[:, t * VW:(t + 1) * VW].rearrange(
                        "p (h w) -> p h w", h=H
                    )[:, :, 0:D],
                    in_=vs[:].rearrange("p (h d) -> p h d", h=H),
                )

            # proj psum: two [128, 512] tiles hold the four [128, 256]
            # tq-chunk partials across both passes
            pjt = [pjp.tile([128, 2 * E], F32, name=f"pjt{i}", tag="pj")
                   for i in range(2)]

            # ---------- phase 2: attention (2 passes of 4 heads) ----------
            # PV uses P.T chunks as the STATIONARY operand and [v|1] as the
            # moving operand, so O lands in natural [tq, d] orientation with
            # the softmax denominator Z as a per-partition column. The O.T
            # needed by the projection comes from xbar DMA transposes.
            o_sb = []
            for p in range(NPASS):
                osb = osbp.tile([128, TQ], BF16, tag="osb")
                o_sb.append(osb)
                # o_nat: [tq(4x128), 4 heads x 32] natural-orientation output
                onat = osbp.tile([128, TQ], BF16, tag="onat")
                for u in range(4):  # heads within pass
                    h = p * 4 + u
                    hc, hr = h // 2, (h % 2) * D
                    # po: per-tqc [O_h | Z] blocks: [128, 4*33]
                    po = otp.tile([128, 4 * (D + 1)], F32, tag="po")
                    for g in range(NTK // 2):
                        st = stp.tile([128, 1024], F32, tag="st")
                        for i in range(2):
                            tk = 2 * g + i
                            nc.tensor.matmul(
                                st[:, i * 512:(i + 1) * 512],
                                kT[hc][hr:hr + D, tk * 128:(tk + 1) * 128],
                                qT[hc][hr:hr + D, :],
                                start=True,
                                stop=True,
                            )
                        pt = ptp.tile([128, 1024], BF16, tag="pt")
                        nc.scalar.activation(pt[:], st[:], AF.Exp, scale=SCALE)
                        for i in range(2):
                            tk = 2 * g + i
                            for m in range(4):  # tq 128-chunks
                                # start=True zeroes the whole 2KB psum bank
                                # row, so only the first matmul starts the
                                # group and only the last one stops it.
                                nc.tensor.matmul(
                                    po[:, m * (D + 1):(m + 1) * (D + 1)],
                                    pt[:, i * 512 + m * 128: i * 512 + (m + 1) * 128],
                                    v_buf[:, tk * VW + h * (D + 1): tk * VW + (h + 1) * (D + 1)],
                                    start=(g == 0 and i == 0 and m == 0),
                                    stop=(g == NTK // 2 - 1 and i == 1 and m == 3),
                                    skip_group_check=True,
                                )

                    # normalize head h: per-partition 1/Z then scalar mul
                    rz = rzp.tile([128, 4], F32, tag="rz")
                    nc.vector.reciprocal(
                        rz[:], po[:].rearrange("p (m w) -> p m w", w=D + 1)[:, :, D]
                    )
                    for m in range(4):
                        nc.vector.tensor_scalar_mul(
                            onat[:, m * 128 + u * D: m * 128 + (u + 1) * D],
                            po[:, m * (D + 1): m * (D + 1) + D],
                            rz[:, m:m + 1],
                        )

                # o_sb[:, m*128:+128] = transpose(o_nat[:, m*128:+128]):
                # [tq 128, 4 heads x 32] -> [4 heads x 32, tq 128]
                for m in range(4):
                        eng = nc.sync if m % 2 == 0 else nc.scalar
                        eng.dma_start_transpose(
                            osb[:, m * 128:(m + 1) * 128],
                            onat[:, m * 128:(m + 1) * 128],
                        )

                # accumulate this pass's projection contribution
                # (pass 0's matmuls overlap pass 1's attention compute)
                for m in range(4):
                    nc.tensor.matmul(
                        pjt[m // 2][:, (m % 2) * E:(m % 2 + 1) * E],
                        osb[:, m * 128:(m + 1) * 128],
                        w_sb["p"][:, p * E:(p + 1) * E],
                        start=(p == 0 and m % 2 == 0),
                        stop=(p == NPASS - 1 and m % 2 == 1),
                        skip_group_check=True,
                    )

            # ---------- phase 3: bias + DMA out (proj accumulated
            # inside the pass loop above) ----------
            for m in range(TQ // 128):
                ob = outp.tile([128, E], F32, tag="outsb")
                nc.vector.tensor_tensor(
                    ob[:], pjt[m // 2][:, (m % 2) * E:(m % 2 + 1) * E],
                    bp_sb[:], ALU.add,
                )
                eng = nc.sync if m % 2 == 0 else nc.scalar
                eng.dma_start(
                    out=out[m * 128:(m + 1) * 128, :], in_=ob[:]
                )

    return nc


def get_graph():
    global _GRAPH
    if _GRAPH is None:
        nc = _build_graph()
        nc.compile()
        _GRAPH = nc
    return _GRAPH


def make_in_maps(query, key_, value, Wq, bq, Wk, bk, Wv, bv, Wp, bp):
    query = np.asarray(query, np.float32)
    key_ = np.asarray(key_, np.float32)
    value = np.asarray(value, np.float32)
    Wq, Wk, Wv, Wp = (np.asarray(w, np.float32) for w in (Wq, Wk, Wv, Wp))
    bq, bk, bv, bp = (np.asarray(b_, np.float32) for b_ in (bq, bk, bv, bp))

    wq_b = np.ascontiguousarray(Wq).astype(BF)
    wk_b = np.ascontiguousarray(Wk).astype(BF)
    wv_b = np.ascontiguousarray(Wv).astype(BF)
    wp_b = np.ascontiguousarray(Wp).astype(BF)
    xt = {}
    for b in range(B):
        xt[("q", b)] = np.ascontiguousarray(query[:, b, :].T).astype(BF)
        xt[("k", b)] = np.ascontiguousarray(key_[:, b, :].T).astype(BF)
        xt[("v", b)] = np.ascontiguousarray(value[:, b, :].T).astype(BF)

    in_maps = []
    for c in range(NCORES):
        b = c // SP
        p = c % SP
        m = {
            "xqt": np.ascontiguousarray(xt[("q", b)][:, p * TQ:(p + 1) * TQ]),
            "xkt": xt[("k", b)],
            "xvt": xt[("v", b)],
            "wq": wq_b,
            "wk": wk_b,
            "wv": wv_b,
            "wp": wp_b,
            "bq": bq.reshape(1, E).copy(),
            "bk": bk.reshape(1, E).copy(),
            "bv": bv.reshape(1, E).copy(),
            "bp": bp.reshape(1, E).copy(),
        }
        in_maps.append(m)
    return in_maps


def assemble(results):
    out_full = np.empty((L, B, E), np.float32)
    for c in range(NCORES):
        b = c // SP
        p = c % SP
        out_full[p * TQ:(p + 1) * TQ, b, :] = results[c]["out"]
    return out_full


def run(inputs, trace=False, **kw):
    nc = get_graph()
    in_maps = make_in_maps(**inputs)
    res = run_bass_kernel_spmd(
        nc, in_maps, core_ids=list(range(NCORES)), trace=trace, **kw
    )
    return res


def kernel(**inputs):
    res = run(inputs, trace=False)
    return assemble(res.results)
